# revision 6
# baseline (speedup 1.0000x reference)
"""Trainium2 Bass kernel for nn_MultiHeadAttention_78520592105487.

MultiHeadAttention: B=2, S=2048, D=1024, H=16, head_dim=64, causal +
key-padding masks, post-softmax query-padding zeroing, residual add.

Sharding (8 cores): core c handles batch b=c//4 and head group g=c%4
(4 heads, 256 output columns). QKV projection weights column-sharded by
head; q/k/v activations replicated per batch.

Per-core device program (bf16 matmuls, f32 accumulation):
  - Projections computed transposed: QT/KT = [d_head-major, seq] layout so
    attention scores S^T = KT_h.T @ QT_h come out [keys, queries] with no
    on-chip transposes. V computed [seq, d] layout with a ones column
    appended per head (denominator trick).
  - exp via ScalarE with the 1/sqrt(64) scale and the key-padding additive
    mask (0/-1e10 per key = per-partition bias) fused into the activation.
  - causal mask applied multiplicatively post-exp on diagonal tiles only
    (4 static [128,512] patterns, generated on device).
  - PV: out^T accumulated in PSUM over key tiles; row 64 = softmax
    denominator. PE-transpose [65,128] blocks -> [128,65], then normalize
    (per-partition reciprocal x query mask), add residual, store.
"""

import sys
import numpy as np

for _p in ("/opt/trn_rl_repo", "/root/.axon_site/_ro/trn_rl_repo"):
    if _p not in sys.path:
        sys.path.insert(0, _p)

B, S, D, H = 2, 2048, 1024, 16
HD = D // H            # 64
NCORES = 8
GH = 4                 # heads per core
CW = GH * HD           # 256 output cols per core
KT_TILES = S // 128    # 16 key tiles of 128
QT_TILES = S // 512    # 4 query tiles of 512
KD = D // 128          # 8 contraction tiles

_CACHE = {}


def _build_program():
    import concourse.bass as bass
    import concourse.mybir as mybir
    from concourse.tile import TileContext
    from concourse.masks import make_identity

    dt = mybir.dt
    F32, BF16 = dt.float32, dt.bfloat16
    AF = mybir.ActivationFunctionType

    nc = bass.Bass()
    qT = nc.declare_dram_parameter("qT", [D, S], BF16, isOutput=False)
    kT = nc.declare_dram_parameter("kT", [D, S], BF16, isOutput=False)
    vT = nc.declare_dram_parameter("vT", [D, S], BF16, isOutput=False)
    wqT = nc.declare_dram_parameter("wqT", [D, CW], BF16, isOutput=False)
    wkT = nc.declare_dram_parameter("wkT", [D, CW], BF16, isOutput=False)
    wvT = nc.declare_dram_parameter("wvT", [D, CW], BF16, isOutput=False)
    bqc = nc.declare_dram_parameter("bqc", [128, 2], F32, isOutput=False)
    bkc = nc.declare_dram_parameter("bkc", [128, 2], F32, isOutput=False)
    bvc = nc.declare_dram_parameter("bvc", [1, CW], F32, isOutput=False)
    kb = nc.declare_dram_parameter("kb", [128, KT_TILES], F32, isOutput=False)
    qm = nc.declare_dram_parameter("qm", [128, KT_TILES], F32, isOutput=False)
    qres = nc.declare_dram_parameter("qres", [S, CW], F32, isOutput=False)
    out = nc.declare_dram_parameter("out", [S, CW], F32, isOutput=True)

    with TileContext(nc) as tc:
        with tc.tile_pool(name="const", bufs=1) as const, \
             tc.tile_pool(name="ins", bufs=1) as ins, \
             tc.tile_pool(name="acts", bufs=1) as acts, \
             tc.tile_pool(name="work", bufs=2) as work, \
             tc.tile_pool(name="epi", bufs=3) as epi, \
             tc.tile_pool(name="ps", bufs=2, space="PSUM") as ps, \
             tc.tile_pool(name="psacc", bufs=1, space="PSUM") as psacc:

            # ---- constants ----
            ident = const.tile([128, 128], F32, tag="ident")
            make_identity(nc, ident)

            # causal patterns: cm[:, i, y] keeps (valid) where y - x - 128*i >= 0
            cmask = const.tile([128, 4, 512], BF16, tag="cmask")
            for i in range(4):
                nc.gpsimd.memset(cmask[:, i, :], 1.0)
                nc.gpsimd.affine_select(
                    out=cmask[:, i, :], in_=cmask[:, i, :],
                    compare_op=mybir.AluOpType.is_ge, fill=0.0,
                    base=-128 * i, channel_multiplier=-1, pattern=[[1, 512]],
                )

            kb_sb = const.tile([128, KT_TILES], F32, tag="kb")
            nc.sync.dma_start(out=kb_sb, in_=kb[:, :])
            qm_sb = const.tile([128, KT_TILES], F32, tag="qm")
            nc.sync.dma_start(out=qm_sb, in_=qm[:, :])
            bq_sb = const.tile([128, 2], F32, tag="bq")
            nc.sync.dma_start(out=bq_sb, in_=bqc[:, :])
            bk_sb = const.tile([128, 2], F32, tag="bk")
            nc.sync.dma_start(out=bk_sb, in_=bkc[:, :])
            bv_sb = const.tile([128, CW], F32, tag="bv")
            _bv_ap = bvc.ap() if hasattr(bvc, "ap") else bvc[:, :]
            bv_bcast = bass.AP(tensor=_bv_ap.tensor, offset=_bv_ap.offset,
                               ap=[[0, 128], [1, CW]])
            nc.gpsimd.dma_start(out=bv_sb, in_=bv_bcast)

            out_stage = const.tile([128, KT_TILES, CW], F32, tag="ostage")
            qres_sb = const.tile([128, KT_TILES, CW], F32, tag="qres")
            for a in range(KT_TILES):
                nc.sync.dma_start(out=qres_sb[:, a, :],
                                  in_=qres[a * 128:(a + 1) * 128, :])

            # ---- bulk inputs ----
            qT_sb = ins.tile([128, KD, S], BF16, tag="qT")
            kT_sb = ins.tile([128, KD, S], BF16, tag="kT")
            vT_sb = ins.tile([128, KD, S], BF16, tag="vT")
            wq_sb = ins.tile([128, KD, CW], BF16, tag="wq")
            wk_sb = ins.tile([128, KD, CW], BF16, tag="wk")
            wv_sb = ins.tile([128, KD, CW], BF16, tag="wv")
            for a in range(KD):
                nc.sync.dma_start(out=wq_sb[:, a, :], in_=wqT[a * 128:(a + 1) * 128, :])
                nc.sync.dma_start(out=wk_sb[:, a, :], in_=wkT[a * 128:(a + 1) * 128, :])
                nc.sync.dma_start(out=wv_sb[:, a, :], in_=wvT[a * 128:(a + 1) * 128, :])
                nc.sync.dma_start(out=qT_sb[:, a, :], in_=qT[a * 128:(a + 1) * 128, :])
                nc.sync.dma_start(out=kT_sb[:, a, :], in_=kT[a * 128:(a + 1) * 128, :])
                nc.sync.dma_start(out=vT_sb[:, a, :], in_=vT[a * 128:(a + 1) * 128, :])

            # ---- projections ----
            # QT/KT: [dout(=256, 2 m-tiles of 128), seq] = W_c @ x^T
            QT_sb = acts.tile([128, 2, S], BF16, tag="QT")
            KT_sb = acts.tile([128, 2, S], BF16, tag="KT")
            for dst, w_sb, x_sb, b_sb in ((QT_sb, wq_sb, qT_sb, bq_sb),
                                          (KT_sb, wk_sb, kT_sb, bk_sb)):
                for m in range(2):
                    for n in range(QT_TILES):
                        pacc = ps.tile([128, 512], F32, tag="sa" if m == 0 else "sb")
                        for a in range(KD):
                            nc.tensor.matmul(
                                pacc,
                                lhsT=w_sb[:, a, m * 128:(m + 1) * 128],
                                rhs=x_sb[:, a, n * 512:(n + 1) * 512],
                                start=(a == 0), stop=(a == KD - 1),
                            )
                        nc.scalar.activation(
                            out=dst[:, m, n * 512:(n + 1) * 512], in_=pacc,
                            func=AF.Identity, bias=b_sb[:, m:m + 1], scale=1.0,
                        )

            # V: [seq(16 tiles), 4 heads x 65] with ones column per head
            V_sb = acts.tile([128, KT_TILES, GH, HD + 1], BF16, tag="V")
            for s in range(KT_TILES):
                pv = ps.tile([128, CW], F32, tag="tr")
                for a in range(KD):
                    nc.tensor.matmul(
                        pv,
                        lhsT=vT_sb[:, a, s * 128:(s + 1) * 128],
                        rhs=wv_sb[:, a, :],
                        start=(a == 0), stop=(a == KD - 1),
                    )
                pv_v = pv.rearrange("p (h d) -> p h d", h=GH)
                bv_v = bv_sb.rearrange("p (h d) -> p h d", h=GH)
                nc.vector.tensor_add(V_sb[:, s, :, 0:HD], pv_v, bv_v)
                nc.vector.memset(V_sb[:, s, :, HD:HD + 1], 1.0)

            # ---- attention ----
            for hp in range(2):          # head pair: local heads 2hp, 2hp+1
                for qj in range(QT_TILES):
                    nk = 4 * qj + 4      # causal: key tiles 0..4qj+3
                    oA = psacc.tile([HD + 1, 512], F32, tag="oa")
                    oB = psacc.tile([HD + 1, 512], F32, tag="ob")
                    for ki in range(nk):
                        sA = ps.tile([128, 512], F32, tag="sa")
                        sB = ps.tile([128, 512], F32, tag="sb")
                        nc.tensor.matmul(
                            sA, lhsT=KT_sb[0:64, hp, ki * 128:(ki + 1) * 128],
                            rhs=QT_sb[0:64, hp, qj * 512:(qj + 1) * 512],
                            start=True, stop=True)
                        nc.tensor.matmul(
                            sB, lhsT=KT_sb[64:128, hp, ki * 128:(ki + 1) * 128],
                            rhs=QT_sb[64:128, hp, qj * 512:(qj + 1) * 512],
                            start=True, stop=True)
                        pta = work.tile([128, 512], BF16, tag="pta")
                        ptb = work.tile([128, 512], BF16, tag="ptb")
                        nc.scalar.activation(out=pta, in_=sA, func=AF.Exp,
                                             bias=kb_sb[:, ki:ki + 1], scale=0.125)
                        nc.scalar.activation(out=ptb, in_=sB, func=AF.Exp,
                                             bias=kb_sb[:, ki:ki + 1], scale=0.125)
                        if ki >= 4 * qj:        # diagonal block: causal wedge
                            i = ki - 4 * qj
                            nc.vector.tensor_mul(pta, pta, cmask[:, i, :])
                            nc.vector.tensor_mul(ptb, ptb, cmask[:, i, :])
                        nc.tensor.matmul(oA, lhsT=V_sb[:, ki, 2 * hp, :], rhs=pta,
                                         start=(ki == 0), stop=(ki == nk - 1))
                        nc.tensor.matmul(oB, lhsT=V_sb[:, ki, 2 * hp + 1, :], rhs=ptb,
                                         start=(ki == 0), stop=(ki == nk - 1))
                    for h, o_ps in ((2 * hp, oA), (2 * hp + 1, oB)):
                        po = work.tile([HD + 1, 512], F32, tag="po")
                        nc.vector.tensor_copy(po, o_ps)
                        for sub in range(4):
                            qi = qj * 4 + sub
                            tr = ps.tile([128, HD + 1], F32, tag="tr")
                            nc.tensor.transpose(
                                tr, po[:, sub * 128:(sub + 1) * 128],
                                ident[0:HD + 1, 0:HD + 1])
                            rc = epi.tile([128, 1], F32, tag="rc")
                            nc.vector.reciprocal(rc, tr[:, HD:HD + 1])
                            rm = epi.tile([128, 1], F32, tag="rm")
                            nc.vector.tensor_mul(rm, rc, qm_sb[:, qi:qi + 1])
                            tm = epi.tile([128, HD], F32, tag="tm")
                            nc.vector.tensor_scalar_mul(tm, tr[:, 0:HD], rm)
                            nc.vector.tensor_add(
                                out_stage[:, qi, h * HD:(h + 1) * HD],
                                tm, qres_sb[:, qi, h * HD:(h + 1) * HD])

            for a in range(KT_TILES):
                nc.sync.dma_start(out=out[a * 128:(a + 1) * 128, :],
                                  in_=out_stage[:, a, :])

    _split_wide_drains(nc, mybir)
    return nc


def _split_wide_drains(nc, mybir, maxw=1):
    """This walrus build rejects instructions carrying more than ~1-2
    embedded sem waits. Hoist excess waits onto single-wait NoOps placed
    immediately before the instruction on the same engine (the engine's
    instruction stream is sequential, so semantics are preserved)."""
    for f in nc.m.functions:
        for blk in f.blocks:
            newlist = []
            for inst in blk.instructions:
                si = inst.sync_info
                if si is not None and si.on_wait and len(si.on_wait) > maxw:
                    waits = list(si.on_wait)
                    head, tail = waits[:-maxw], waits[-maxw:]
                    for k, w in enumerate(head):
                        pre = mybir.InstDrain(
                            name=f"{inst.name}-presplit{k}",
                            engine=inst.engine, ins=[], outs=[],
                            sync_info=mybir.SyncInfo(on_wait=[w], on_update=[]),
                        )
                        nc.register_instruction(pre)
                        newlist.append(pre)
                    inst.sync_info = mybir.SyncInfo(
                        on_wait=tail, on_update=list(si.on_update or []))
                newlist.append(inst)
            blk.instructions = newlist


def _prep_in_maps(q, k, v, Wq, bq, Wk, bk, Wv, bv, key_padding_mask,
                  query_padding_mask):
    import ml_dtypes
    BF = ml_dtypes.bfloat16
    f32 = np.float32

    q = np.asarray(q, f32)
    k = np.asarray(k, f32)
    v = np.asarray(v, f32)
    Wq, Wk, Wv = (np.asarray(w, f32) for w in (Wq, Wk, Wv))
    bq, bk, bv = (np.asarray(b_, f32) for b_ in (bq, bk, bv))
    kmask = np.asarray(key_padding_mask, bool)
    qmask = np.asarray(query_padding_mask, bool)

    qT = [np.ascontiguousarray(q[b].T).astype(BF) for b in range(B)]
    kT = [np.ascontiguousarray(k[b].T).astype(BF) for b in range(B)]
    vT = [np.ascontiguousarray(v[b].T).astype(BF) for b in range(B)]
    kb_all = [np.ascontiguousarray(
        np.where(kmask[b], 0.0, -1e10).astype(f32).reshape(KT_TILES, 128).T)
        for b in range(B)]
    qm_all = [np.ascontiguousarray(
        qmask[b].astype(f32).reshape(KT_TILES, 128).T) for b in range(B)]

    in_maps = []
    for c in range(NCORES):
        b = c // 4
        g = c % 4
        sl = slice(CW * g, CW * (g + 1))
        in_maps.append({
            "qT": qT[b], "kT": kT[b], "vT": vT[b],
            "wqT": np.ascontiguousarray(Wq[sl].T).astype(BF),
            "wkT": np.ascontiguousarray(Wk[sl].T).astype(BF),
            "wvT": np.ascontiguousarray(Wv[sl].T).astype(BF),
            "bqc": np.ascontiguousarray(bq[sl].reshape(2, 128).T),
            "bkc": np.ascontiguousarray(bk[sl].reshape(2, 128).T),
            "bvc": np.ascontiguousarray(bv[sl].reshape(1, CW)),
            "kb": kb_all[b], "qm": qm_all[b],
            "qres": np.ascontiguousarray(q[b][:, sl]),
        })
    return in_maps


def kernel(**inputs):
    from concourse.bass_utils import run_bass_kernel_spmd

    if "prog" not in _CACHE:
        _CACHE["prog"] = _build_program()
    nc = _CACHE["prog"]

    in_maps = _prep_in_maps(**inputs)
    res = run_bass_kernel_spmd(nc, in_maps, core_ids=list(range(NCORES)))

    full = np.empty((B, S, D), np.float32)
    for c in range(NCORES):
        b = c // 4
        g = c % 4
        full[b, :, CW * g:CW * (g + 1)] = res.results[c]["out"]
    return full
